# revision 23
# baseline (speedup 1.0000x reference)
"""Trainium2 Bass kernel for nn_Conv2d_24833500905755 (3x3 conv, B=32,
C_in=64, C_out=128, 56x56, pad 1, with the reference's mismatched
weight-flatten order).

Math: out[b,co,h,w] = sum_{c,di,dj} xpad[b,c,h+di,w+dj] * Wt[c,di*3+dj,co]
with Wt = K.reshape(576, C_OUT).reshape(C_IN, 9, C_OUT).

Data-parallel: 4 images per NeuronCore, 2 images packed on the
128-partition dim (fp16 matmuls, K=64 contraction per half, concurrent
PE row-group tiles). Raw-bass hand-scheduled engine programs.

v4 schedule notes (from trace analysis):
- DMA completion semaphores are incremented one per engine-slice (16 per
  DMA) and the last 1-2 slices straggle by 1-3us behind the bulk of the
  data. Gates are therefore per queue-half (16 slices, one queue) and the
  h1 (scalar-queue) image stream runs one chunk BEHIND h0, so h0 starts
  on the sync queue's data while h1's stragglers land.
- The PE clock needs ~4.6us of CONTINUOUS matmul activity to ramp to
  8/8; warm-up junk pairs (~373ns each at the slow clock) bridge from
  engine start (~7.2us) to the first gate release (~10.4us). They read a
  dedicated junk SBUF tensor so their SBUF traffic cannot collide with
  the input DMA writes.
- Junk pairs after the last real matmul keep the clock up through the
  output-DMA tail and the NEFF postamble's semaphore-reset chains.
- No trailing all-engine barrier (the postamble rendezvous is enough);
  only sync waits for output-DMA completion.

  Sync:   h0 input DMAs, h0 output batch DMAs, final s_out wait
  Scalar: h1 input DMAs, h1 PSUM->SBUF copies, h1 output batch DMAs
  Tensor: warm-up junk + skewed h0/h1 matmul streams + tail junk
  Vector: h0 PSUM->SBUF copies
"""

from contextlib import ExitStack

import numpy as np

import concourse.bass as bass
import concourse.mybir as mybir
from concourse.bass import BassBlock
from concourse.bass_utils import run_bass_kernel_spmd

B, C_IN, C_OUT, H = 32, 64, 128, 56
KS = 3
N_CORES = 8
BPC = B // N_CORES
HP = H + 2
RCHUNK = 8
NCHUNK = H // RCHUNK          # 7 chunks/image
NCH = 2 * NCHUNK              # 14 chunks per half across both pairs
MM_DT = mybir.dt.float16

# x row pieces per pair-0 image: piece i covers rows [XPIECES[i], XPIECES[i+1])
XPIECES = [0, 10, 34, HP]
# chunk ci needs input rows <= ci*8+10; piece gate index per chunk
CHUNK_PIECE = [0, 1, 1, 1, 2, 2, 2]
# output batches (row ranges) per image; finer at the end so the tail
# drains fast
OBATCH = [(0, 16), (16, 32), (32, 40), (40, 48), (48, 56)]
N_OUT_DMAS = BPC * len(OBATCH)
N_WARMUP_PAIRS = 8
WKSPLIT = 5  # w[:, 0:5, :] gates k<5; w[:, 5:9, :] gates the rest


class NoBarrierBlock(BassBlock):
    """BassBlock without the exit-time all-engine barrier/drain: the
    compiler-emitted postamble performs its own rendezvous before the
    final semaphore teardown, so the extra barrier only adds latency."""

    def __exit__(self, exc_type, exc_val, exc_tb):
        if exc_type is None:
            for engine, last_body in self.last_body.items():
                with self.bass.body(
                    last_body, parent=self.bass.cur_bb, allow_existing_parent=True
                ):
                    engine.br(self.end_bb)
            self.bass.switch_bb(self.end_bb)


def build_nc(mm_dt=MM_DT):
    f32 = mybir.dt.float32
    nc = bass.Bass()
    x_ext = nc.declare_dram_parameter("x", [BPC, C_IN, HP, HP], mm_dt, isOutput=False)
    w_ext = nc.declare_dram_parameter("w", [2 * C_IN, KS * KS, C_OUT], mm_dt, isOutput=False)
    out_ext = nc.declare_dram_parameter("out", [BPC, C_OUT, H, H], f32, isOutput=True)

    with ExitStack() as ctx:
        wt = ctx.enter_context(nc.sbuf_tensor("wt", [2 * C_IN, KS * KS, C_OUT], mm_dt))
        xps = [
            ctx.enter_context(nc.sbuf_tensor(f"xp{p}", [2 * C_IN, HP, HP], mm_dt))
            for p in range(2)
        ]
        junk = ctx.enter_context(
            nc.sbuf_tensor("junk", [2 * C_IN, RCHUNK, C_OUT], mm_dt)
        )
        obs = [
            ctx.enter_context(nc.sbuf_tensor(f"ob{b}", [C_OUT, H, H], f32))
            for b in range(BPC)
        ]
        banks = [
            [
                ctx.enter_context(
                    nc.psum_tensor(f"ps_{s}_{h}", [C_OUT, RCHUNK, H], f32)
                )
                for h in range(2)
            ]
            for s in range(4)
        ]
        # input gates: each inc'd by one single-queue DMA (full at >=16);
        # gating DMAs go FIRST in their queue - completion slices straggle
        # badly behind earlier descriptors on the same queue
        s_wa = ctx.enter_context(nc.semaphore("s_wa"))
        s_wb = ctx.enter_context(nc.semaphore("s_wb"))
        s_p0 = [ctx.enter_context(nc.semaphore(f"s_p0h{h}")) for h in range(2)]
        s_xp = [
            [ctx.enter_context(nc.semaphore(f"s_xp{i}h{h}")) for h in range(2)]
            for i in range(2)  # piece 1, piece 2
        ]
        s_x1 = [ctx.enter_context(nc.semaphore(f"s_x1h{h}")) for h in range(2)]
        s_mm = [ctx.enter_context(nc.semaphore(f"s_mm{h}")) for h in range(2)]
        s_cpv = ctx.enter_context(nc.semaphore("s_cpv"))   # h0 copies (vector)
        s_cph = ctx.enter_context(nc.semaphore("s_cph"))   # h1 copies (scalar)
        s_out = ctx.enter_context(nc.semaphore("s_out"))

        def in_dmas(eng, h):
            """Input DMAs for half h's queue: piece0_h first, then a k-range
            of w for BOTH halves (h0 queue: k<WKSPLIT, h1 queue: rest), then
            this half's remaining row pieces."""
            c0 = h * C_IN
            src0 = x_ext[h : h + 1].rearrange("b c h w -> (b c) h w")
            src1 = x_ext[2 + h : 3 + h].rearrange("b c h w -> (b c) h w")
            lo, hi = XPIECES[0], XPIECES[1]
            eng.dma_start(
                out=xps[0][c0 : c0 + C_IN, lo:hi, :], in_=src0[:, lo:hi, :]
            ).then_inc(s_p0[h], 16)
            if h == 0:
                eng.dma_start(
                    out=wt[:, 0:WKSPLIT, :], in_=w_ext[:, 0:WKSPLIT, :]
                ).then_inc(s_wa, 16)
            else:
                eng.dma_start(
                    out=wt[:, WKSPLIT:, :], in_=w_ext[:, WKSPLIT:, :]
                ).then_inc(s_wb, 16)
            for i in range(2):
                lo, hi = XPIECES[i + 1], XPIECES[i + 2]
                eng.dma_start(
                    out=xps[0][c0 : c0 + C_IN, lo:hi, :], in_=src0[:, lo:hi, :]
                ).then_inc(s_xp[i][h], 16)
            eng.dma_start(
                out=xps[1][c0 : c0 + C_IN, :, :], in_=src1[:, :, :]
            ).then_inc(s_x1[h], 16)

        # global chunk list: g -> (pair, chunk-in-image)
        chunks = [(p, ci) for p in range(2) for ci in range(NCHUNK)]

        def chunk_waits(tensor, g):
            p, ci = chunks[g]
            if p == 0:
                if ci == 0:
                    tensor.wait_ge(s_wa, 16)
                    tensor.wait_ge(s_p0[0], 16)
                    tensor.wait_ge(s_p0[1], 16)
                elif CHUNK_PIECE[ci] > CHUNK_PIECE[ci - 1]:
                    pi = CHUNK_PIECE[ci] - 1
                    tensor.wait_ge(s_xp[pi][0], 16)
                    tensor.wait_ge(s_xp[pi][1], 16)
            else:
                if ci == 0:
                    tensor.wait_ge(s_x1[0], 16)
                    tensor.wait_ge(s_x1[1], 16)
            if g >= 4:
                # WAR: bank slot g%4 last written by chunk g-4
                tensor.wait_ge(s_cpv, g - 3)
                tensor.wait_ge(s_cph, g - 3)

        def chunk_mm(tensor, h, g, k):
            p, ci = chunks[g]
            h0r = ci * RCHUNK
            di, dj = divmod(k, KS)
            c0 = h * C_IN
            return tensor.matmul(
                out=banks[g % 4][h][:],
                lhsT=wt[c0 : c0 + C_IN, k, :],
                rhs=xps[p][c0 : c0 + C_IN, h0r + di : h0r + di + RCHUNK, dj : dj + H],
                start=(k == 0),
                stop=(k == KS * KS - 1),
            )

        with NoBarrierBlock(nc, "blk") as block:

            @block.sync
            def _(sync: bass.BassEngine):
                in_dmas(sync, 0)
                for p in range(2):
                    img = 2 * p
                    dst = out_ext[img : img + 1].rearrange("b c h w -> (b c) h w")
                    for (blo, bhi) in OBATCH:
                        last_chunk = p * NCHUNK + (bhi - 1) // RCHUNK
                        sync.wait_ge(s_cpv, last_chunk + 1)
                        sync.dma_start(
                            out=dst[:, blo:bhi, :], in_=obs[img][:, blo:bhi, :]
                        ).then_inc(s_out, 16)
                sync.wait_ge(s_out, 16 * N_OUT_DMAS)

            @block.scalar
            def _(scalar: bass.BassEngine):
                in_dmas(scalar, 1)
                for p in range(2):
                    img = 2 * p + 1
                    dst = out_ext[img : img + 1].rearrange("b c h w -> (b c) h w")
                    bi = 0
                    for ci in range(NCHUNK):
                        g = p * NCHUNK + ci
                        h0r = ci * RCHUNK
                        scalar.wait_ge(s_mm[1], g + 1)
                        scalar.copy(
                            out=obs[img][:, h0r : h0r + RCHUNK, :],
                            in_=banks[g % 4][1][:],
                        ).then_inc(s_cph, 1)
                        blo, bhi = OBATCH[bi]
                        if h0r + RCHUNK == bhi:
                            scalar.dma_start(
                                out=dst[:, blo:bhi, :], in_=obs[img][:, blo:bhi, :]
                            ).then_inc(s_out, 16)
                            bi += 1

            @block.tensor
            def _(tensor: bass.BassEngine):
                def junk_pairs(n):
                    # reads/writes buffers no DMA or copy touches (warm-up
                    # SBUF reads must not collide with in-flight input DMAs)
                    for wi in range(2 * n):
                        h = wi % 2
                        c0 = h * C_IN
                        tensor.matmul(
                            out=banks[2][h][:],
                            lhsT=junk[c0 : c0 + C_IN, 0, :],
                            rhs=junk[c0 : c0 + C_IN, :, 0:H],
                            start=True,
                            stop=True,
                        )

                junk_pairs(N_WARMUP_PAIRS)
                # fully-paired stream: the clock ramp only advances under
                # full-PE (both row-group) activity, so solo-half slots are
                # a net loss
                for g in range(NCH):
                    chunk_waits(tensor, g)
                    for k in range(KS * KS):
                        if g == 0 and k == WKSPLIT:
                            tensor.wait_ge(s_wb, 16)
                        last = k == KS * KS - 1
                        for h in range(2):
                            mm = chunk_mm(tensor, h, g, k)
                            if last:
                                mm.then_inc(s_mm[h], 1)

            @block.vector
            def _(vector: bass.BassEngine):
                for p in range(2):
                    img = 2 * p
                    for ci in range(NCHUNK):
                        g = p * NCHUNK + ci
                        h0r = ci * RCHUNK
                        vector.wait_ge(s_mm[0], g + 1)
                        vector.tensor_copy(
                            out=obs[img][:, h0r : h0r + RCHUNK, :],
                            in_=banks[g % 4][0][:],
                        ).then_inc(s_cpv, 1)

    return nc


def _prep_inputs(x, K, mm_dt=MM_DT):
    np_dt = mybir.dt.np(mm_dt)
    x = np.ascontiguousarray(np.asarray(x, dtype=np.float32))
    K = np.ascontiguousarray(np.asarray(K, dtype=np.float32))
    xpad = np.pad(x, ((0, 0), (0, 0), (1, 1), (1, 1))).astype(np_dt)
    Wt = K.reshape(KS * KS * C_IN, C_OUT).reshape(C_IN, KS * KS, C_OUT)
    Wrep = np.ascontiguousarray(np.concatenate([Wt, Wt], axis=0)).astype(np_dt)
    shards = xpad.reshape(N_CORES, BPC, C_IN, HP, HP)
    return [{"x": np.ascontiguousarray(shards[i]), "w": Wrep} for i in range(N_CORES)]


def run(x, K, trace=False, mm_dt=MM_DT):
    nc = build_nc(mm_dt)
    in_maps = _prep_inputs(x, K, mm_dt)
    res = run_bass_kernel_spmd(nc, in_maps, list(range(N_CORES)), trace=trace)
    out = np.concatenate([res.results[i]["out"] for i in range(N_CORES)], axis=0)
    return out, res


def kernel(x, K):
    out, _ = run(x, K, trace=False)
    return out


# revision 29
# speedup vs baseline: 1.1147x; 1.1147x over previous
"""Trainium2 Bass kernel for nn_Conv2d_24833500905755 (3x3 conv, B=32,
C_in=64, C_out=128, 56x56, pad 1, with the reference's mismatched
weight-flatten order).

Math: out[b,co,h,w] = sum_{c,di,dj} xpad[b,c,h+di,w+dj] * Wt[c,di*3+dj,co]
with Wt = K.reshape(576, C_OUT).reshape(C_IN, 9, C_OUT).

Data-parallel: 4 images per NeuronCore, 2 images packed on the
128-partition dim (fp16 matmuls, K=64 contraction per half, concurrent
PE row-group tiles). Raw-bass hand-scheduled engine programs.

v4 schedule notes (from trace analysis):
- DMA completion semaphores are incremented one per engine-slice (16 per
  DMA) and the last 1-2 slices straggle by 1-3us behind the bulk of the
  data. Gates are therefore per queue-half (16 slices, one queue) and the
  h1 (scalar-queue) image stream runs one chunk BEHIND h0, so h0 starts
  on the sync queue's data while h1's stragglers land.
- The PE clock needs ~4.6us of CONTINUOUS matmul activity to ramp to
  8/8; warm-up junk pairs (~373ns each at the slow clock) bridge from
  engine start (~7.2us) to the first gate release (~10.4us). They read a
  dedicated junk SBUF tensor so their SBUF traffic cannot collide with
  the input DMA writes.
- Junk pairs after the last real matmul keep the clock up through the
  output-DMA tail and the NEFF postamble's semaphore-reset chains.
- No trailing all-engine barrier (the postamble rendezvous is enough);
  only sync waits for output-DMA completion.

  Sync:   h0 input DMAs, h0 output batch DMAs, final s_out wait
  Scalar: h1 input DMAs, h1 PSUM->SBUF copies, h1 output batch DMAs
  Tensor: warm-up junk + skewed h0/h1 matmul streams + tail junk
  Vector: h0 PSUM->SBUF copies
"""

from contextlib import ExitStack

import numpy as np

import concourse.bass as bass
import concourse.mybir as mybir
from concourse.bass import BassBlock
from concourse.bass_utils import run_bass_kernel_spmd

B, C_IN, C_OUT, H = 32, 64, 128, 56
KS = 3
N_CORES = 8
BPC = B // N_CORES
HP = H + 2
RCHUNK = 8
NCHUNK = H // RCHUNK          # 7 chunks/image
NCH = 2 * NCHUNK              # 14 chunks per half across both pairs
MM_DT = mybir.dt.float16

# x row pieces per pair-0 image: piece i covers rows [XPIECES[i], XPIECES[i+1])
XPIECES = [0, 10, 34, HP]
# chunk ci needs input rows <= ci*8+10; piece gate index per chunk
CHUNK_PIECE = [0, 1, 1, 1, 2, 2, 2]
# output batches (row ranges) per image; finer at the end so the tail
# drains fast
# output batches: pair-0 images mid-stream (coarse), pair-1 images end with
# 4-row slivers so the final DMA leaves as soon as possible after the last
# PSUM copy
OBATCH0 = [(0, 16), (16, 32), (32, 48), (48, 56)]
OBATCH1 = [(0, 16), (16, 32), (32, 40), (40, 48), (48, 52), (52, 56)]
N_OUT_DMAS = 2 * (len(OBATCH0) + len(OBATCH1))
N_WARMUP_PAIRS = 12


class NoBarrierBlock(BassBlock):
    """BassBlock without the exit-time all-engine barrier/drain: the
    compiler-emitted postamble performs its own rendezvous before the
    final semaphore teardown, so the extra barrier only adds latency."""

    def __exit__(self, exc_type, exc_val, exc_tb):
        if exc_type is None:
            for engine, last_body in self.last_body.items():
                with self.bass.body(
                    last_body, parent=self.bass.cur_bb, allow_existing_parent=True
                ):
                    engine.br(self.end_bb)
            self.bass.switch_bb(self.end_bb)


def build_nc(mm_dt=MM_DT):
    f32 = mybir.dt.float32
    nc = bass.Bass()
    x_ext = nc.declare_dram_parameter("x", [BPC, C_IN, HP, HP], mm_dt, isOutput=False)
    w_ext = nc.declare_dram_parameter("w", [2 * C_IN, KS * KS, C_OUT], mm_dt, isOutput=False)
    out_ext = nc.declare_dram_parameter("out", [BPC, C_OUT, H, H], f32, isOutput=True)

    with ExitStack() as ctx:
        wt = ctx.enter_context(nc.sbuf_tensor("wt", [2 * C_IN, KS * KS, C_OUT], mm_dt))
        xps = [
            ctx.enter_context(nc.sbuf_tensor(f"xp{p}", [2 * C_IN, HP, HP], mm_dt))
            for p in range(2)
        ]
        junk = ctx.enter_context(
            nc.sbuf_tensor("junk", [2 * C_IN, RCHUNK, C_OUT], mm_dt)
        )
        obs = [
            ctx.enter_context(nc.sbuf_tensor(f"ob{b}", [C_OUT, H, H], f32))
            for b in range(BPC)
        ]
        banks = [
            [
                ctx.enter_context(
                    nc.psum_tensor(f"ps_{s}_{h}", [C_OUT, RCHUNK, H], f32)
                )
                for h in range(2)
            ]
            for s in range(4)
        ]
        # input gates: each inc'd by one single-queue DMA (full at >=16);
        # gating DMAs go FIRST in their queue - completion slices straggle
        # ~2us behind earlier descriptors on the same queue
        s_w = ctx.enter_context(nc.semaphore("s_w"))
        s_x0p0 = ctx.enter_context(nc.semaphore("s_x0p0"))
        s_xp = [
            [ctx.enter_context(nc.semaphore(f"s_xp{i}h{h}")) for h in range(2)]
            for i in range(2)  # piece 1, piece 2
        ]
        s_x1 = [ctx.enter_context(nc.semaphore(f"s_x1h{h}")) for h in range(2)]
        s_mm = [ctx.enter_context(nc.semaphore(f"s_mm{h}")) for h in range(2)]
        s_cpv = ctx.enter_context(nc.semaphore("s_cpv"))   # h0 copies (vector)
        s_cph = ctx.enter_context(nc.semaphore("s_cph"))   # h1 copies (scalar)
        s_out = ctx.enter_context(nc.semaphore("s_out"))

        def in_dmas_sync(eng):
            """Q1: whole w first (gates chunk 0), then h0 row pieces."""
            src0 = x_ext[0:1].rearrange("b c h w -> (b c) h w")
            src1 = x_ext[2:3].rearrange("b c h w -> (b c) h w")
            eng.dma_start(out=wt[:, :, :], in_=w_ext[:, :, :]).then_inc(s_w, 16)
            for i in range(2):
                lo, hi = XPIECES[i + 1], XPIECES[i + 2]
                eng.dma_start(
                    out=xps[0][0:C_IN, lo:hi, :], in_=src0[:, lo:hi, :]
                ).then_inc(s_xp[i][0], 16)
            eng.dma_start(
                out=xps[1][0:C_IN, :, :], in_=src1[:, :, :]
            ).then_inc(s_x1[0], 16)

        def in_dmas_scalar(eng):
            """Q10: piece0 for BOTH halves first (gates chunk 0), then h1
            row pieces."""
            src0 = x_ext[0:2].rearrange("b c h w -> (b c) h w")
            s0h1 = x_ext[1:2].rearrange("b c h w -> (b c) h w")
            src1 = x_ext[3:4].rearrange("b c h w -> (b c) h w")
            lo, hi = XPIECES[0], XPIECES[1]
            eng.dma_start(
                out=xps[0][:, lo:hi, :], in_=src0[:, lo:hi, :]
            ).then_inc(s_x0p0, 16)
            for i in range(2):
                lo, hi = XPIECES[i + 1], XPIECES[i + 2]
                eng.dma_start(
                    out=xps[0][C_IN:, lo:hi, :], in_=s0h1[:, lo:hi, :]
                ).then_inc(s_xp[i][1], 16)
            eng.dma_start(
                out=xps[1][C_IN:, :, :], in_=src1[:, :, :]
            ).then_inc(s_x1[1], 16)

        # global chunk list: g -> (pair, chunk-in-image)
        chunks = [(p, ci) for p in range(2) for ci in range(NCHUNK)]

        def chunk_waits(tensor, g):
            p, ci = chunks[g]
            if p == 0:
                if ci == 0:
                    tensor.wait_ge(s_w, 16)
                    tensor.wait_ge(s_x0p0, 16)
                elif CHUNK_PIECE[ci] > CHUNK_PIECE[ci - 1]:
                    pi = CHUNK_PIECE[ci] - 1
                    tensor.wait_ge(s_xp[pi][0], 16)
                    tensor.wait_ge(s_xp[pi][1], 16)
            else:
                if ci == 0:
                    tensor.wait_ge(s_x1[0], 16)
                    tensor.wait_ge(s_x1[1], 16)
            if g >= 4:
                # WAR: bank slot g%4 last written by chunk g-4
                tensor.wait_ge(s_cpv, g - 3)
                tensor.wait_ge(s_cph, g - 3)

        def chunk_mm(tensor, h, g, k):
            p, ci = chunks[g]
            h0r = ci * RCHUNK
            di, dj = divmod(k, KS)
            c0 = h * C_IN
            return tensor.matmul(
                out=banks[g % 4][h][:],
                lhsT=wt[c0 : c0 + C_IN, k, :],
                rhs=xps[p][c0 : c0 + C_IN, h0r + di : h0r + di + RCHUNK, dj : dj + H],
                start=(k == 0),
                stop=(k == KS * KS - 1),
            )

        with NoBarrierBlock(nc, "blk") as block:

            # copy-count bookkeeping: copies of chunks g=0..12 count 1..13;
            # the final chunk g=13 is copied as two 4-row halves counting
            # 14 then 15, so the 4-row output slivers can leave early
            def batch_gate(p, bhi):
                last_chunk = p * NCHUNK + (bhi - 1) // RCHUNK
                if p == 1 and bhi > 48:
                    return NCH if bhi == 52 else NCH + 1
                return last_chunk + 1

            @block.sync
            def _(sync: bass.BassEngine):
                in_dmas_sync(sync)
                for p in range(2):
                    img = 2 * p
                    dst = out_ext[img : img + 1].rearrange("b c h w -> (b c) h w")
                    for (blo, bhi) in (OBATCH0 if p == 0 else OBATCH1):
                        sync.wait_ge(s_cpv, batch_gate(p, bhi))
                        sync.dma_start(
                            out=dst[:, blo:bhi, :], in_=obs[img][:, blo:bhi, :]
                        ).then_inc(s_out, 16)
                sync.wait_ge(s_out, 16 * N_OUT_DMAS)

            @block.scalar
            def _(scalar: bass.BassEngine):
                in_dmas_scalar(scalar)
                for p in range(2):
                    img = 2 * p + 1
                    dst = out_ext[img : img + 1].rearrange("b c h w -> (b c) h w")
                    batches = OBATCH0 if p == 0 else OBATCH1
                    bi = 0
                    for ci in range(NCHUNK):
                        g = p * NCHUNK + ci
                        h0r = ci * RCHUNK
                        scalar.wait_ge(s_mm[1], g + 1)
                        if g == NCH - 1:
                            for sub in range(2):
                                r = h0r + 4 * sub
                                scalar.copy(
                                    out=obs[img][:, r : r + 4, :],
                                    in_=banks[g % 4][1][:, 4 * sub : 4 * sub + 4, :],
                                ).then_inc(s_cph, 1)
                                blo, bhi = batches[bi]
                                scalar.dma_start(
                                    out=dst[:, blo:bhi, :],
                                    in_=obs[img][:, blo:bhi, :],
                                ).then_inc(s_out, 16)
                                bi += 1
                        else:
                            scalar.copy(
                                out=obs[img][:, h0r : h0r + RCHUNK, :],
                                in_=banks[g % 4][1][:],
                            ).then_inc(s_cph, 1)
                            blo, bhi = batches[bi]
                            if h0r + RCHUNK == bhi:
                                scalar.dma_start(
                                    out=dst[:, blo:bhi, :], in_=obs[img][:, blo:bhi, :]
                                ).then_inc(s_out, 16)
                                bi += 1

            @block.tensor
            def _(tensor: bass.BassEngine):
                def junk_pairs(n):
                    # reads/writes buffers no DMA or copy touches (warm-up
                    # SBUF reads must not collide with in-flight input DMAs)
                    for wi in range(2 * n):
                        h = wi % 2
                        c0 = h * C_IN
                        tensor.matmul(
                            out=banks[2][h][:],
                            lhsT=junk[c0 : c0 + C_IN, 0, :],
                            rhs=junk[c0 : c0 + C_IN, :, 0:H],
                            start=True,
                            stop=True,
                        )

                junk_pairs(N_WARMUP_PAIRS)
                # fully-paired stream: the clock ramp only advances under
                # full-PE (both row-group) activity, so solo-half slots are
                # a net loss
                for g in range(NCH):
                    chunk_waits(tensor, g)
                    for k in range(KS * KS):
                        last = k == KS * KS - 1
                        for h in range(2):
                            mm = chunk_mm(tensor, h, g, k)
                            if last:
                                mm.then_inc(s_mm[h], 1)

            @block.vector
            def _(vector: bass.BassEngine):
                for p in range(2):
                    img = 2 * p
                    for ci in range(NCHUNK):
                        g = p * NCHUNK + ci
                        h0r = ci * RCHUNK
                        vector.wait_ge(s_mm[0], g + 1)
                        if g == NCH - 1:
                            for sub in range(2):
                                r = h0r + 4 * sub
                                vector.tensor_copy(
                                    out=obs[img][:, r : r + 4, :],
                                    in_=banks[g % 4][0][:, 4 * sub : 4 * sub + 4, :],
                                ).then_inc(s_cpv, 1)
                        else:
                            vector.tensor_copy(
                                out=obs[img][:, h0r : h0r + RCHUNK, :],
                                in_=banks[g % 4][0][:],
                            ).then_inc(s_cpv, 1)

    return nc


def _prep_inputs(x, K, mm_dt=MM_DT):
    np_dt = mybir.dt.np(mm_dt)
    x = np.ascontiguousarray(np.asarray(x, dtype=np.float32))
    K = np.ascontiguousarray(np.asarray(K, dtype=np.float32))
    xpad = np.pad(x, ((0, 0), (0, 0), (1, 1), (1, 1))).astype(np_dt)
    Wt = K.reshape(KS * KS * C_IN, C_OUT).reshape(C_IN, KS * KS, C_OUT)
    Wrep = np.ascontiguousarray(np.concatenate([Wt, Wt], axis=0)).astype(np_dt)
    shards = xpad.reshape(N_CORES, BPC, C_IN, HP, HP)
    return [{"x": np.ascontiguousarray(shards[i]), "w": Wrep} for i in range(N_CORES)]


def run(x, K, trace=False, mm_dt=MM_DT):
    nc = build_nc(mm_dt)
    in_maps = _prep_inputs(x, K, mm_dt)
    res = run_bass_kernel_spmd(nc, in_maps, list(range(N_CORES)), trace=trace)
    out = np.concatenate([res.results[i]["out"] for i in range(N_CORES)], axis=0)
    return out, res


def kernel(x, K):
    out, _ = run(x, K, trace=False)
    return out
